# revision 44
# baseline (speedup 1.0000x reference)
"""Multi-head self-attention Bass kernel for Trainium2, 8 NeuronCores.

Sharding: data-parallel over batch (16 batches -> 2 per core), no collectives;
each core computes full attention for its batches, host gathers.

v3: fully software-pipelined across the two local batches, all-bf16 operands
(f32 PSUM accumulation), engine-balanced (per-pass busy in sim:
PE ~146us, DVE ~142us, ACT ~134us = the exp floor; total ~196us sim,
~245us by test.py's hw_loop differential vs 544us for the v1 baseline):
  - PE: Q^T/K^T projections (lhsT = weight chunks), V projection into
    natural (g, v) layout with an appended ones column per head,
    S^T = K^T q-slice scores (64-contraction, tile_position packs the two
    heads of a pair into PE row halves), natural-layout AV
    (av[q=128, v+1] = u-sliceT @ [V|ones]; only 65 output columns per
    contraction step vs 512 in (v,q) layout), out^T projection.
  - ACT: exp (the floor: 128 x [128,1024] PSUM tiles/pass) + tail copies.
  - DVE: post-exp mask multiply (bf16 2x mode), per-partition reciprocal +
    tensor_scalar normalize (denominator rides in av column 64), PSUM->SBUF
    copies for all projections.
  - DMA engines: [q,hv] -> [hv,q] heads transposes via SBUF-to-SBUF
    XBAR-transpose DMAs (128x128 bf16 blocks), issued from SP/ACT HWDGE.
  - SP (sync): all other DMA issues (HWDGE) - otherwise idle.
Schedule: a work-queue interleaver pops AV/transpose units of the previous
pair (3/chunk) + projection/out units (2/chunk) under the current pair's
exp stream; next-batch projections run under batch b's attention, batch b's
out-projection under b+1's.  PSUM: 2x[128,1024] score tiles + 1 projection
tile + 3 av tiles = 8 banks.  Masking is a post-exp bf16 multiply by keep^T
(== reference's -1e30 additive mask since exp(-1e30)==0); max-subtraction is
skipped (logits bounded, |logit| <= ~23, exp fits f32/bf16).  A ~4us dummy-
matmul warm-up ramps the PE p-state while the first DMAs land.

Emission-order invariant (the Tile framework tracks deps in program order):
a reader emitted before its writer INVERTS the dependency - all consumers of
AV/transpose outputs must be emitted after those units (see the ordered
`work` queue; out-projection units ride it as `late` chunks).
"""
import numpy as np
import ml_dtypes

B, N, D, H, KD = 16, 1024, 512, 8, 64
NCORES = 8
B_LOC = B // NCORES  # 2
P = 128

_NC_CACHE = {}


def build_attention_nc(b_loc=B_LOC, n=N, repeat=1, hw_loop=0, skip=frozenset(),
                       u_bufs=26, uraw_bufs=3, units_per_g=2, qtkt_bufs=6,
                       ps_s_bufs=2, ps_px_bufs=1, ps_av_bufs=3, pool_mask_mod=0,
                       nav=True, work_per_g=3, extra_per_g=2):
    import concourse.bass as bass
    import concourse.mybir as mybir
    import concourse.tile as tile
    from concourse import bacc
    from contextlib import ExitStack
    import contextlib

    F32 = mybir.dt.float32
    BF16 = mybir.dt.bfloat16
    EXP = mybir.ActivationFunctionType.Exp

    d = D
    n_g = n // P          # 128-row key chunks (8)
    n_dc = d // P         # contraction chunks (4)
    n_qh = n // 512       # 512-wide q slices (2)
    n_pairs = H // 2      # head pairs (4)

    nc = bacc.Bacc(trn_type="TRN2", target_bir_lowering=False, debug=False)

    qT_d = nc.dram_tensor("qT", [b_loc, d, n], BF16, kind="ExternalInput").ap()
    mask_d = nc.dram_tensor("maskT", [b_loc, n, n], BF16, kind="ExternalInput").ap()
    wq_d = nc.dram_tensor("wq", [d, d], BF16, kind="ExternalInput").ap()
    wk_d = nc.dram_tensor("wk", [d, d], BF16, kind="ExternalInput").ap()
    wv_d = nc.dram_tensor("wv", [d, d], BF16, kind="ExternalInput").ap()
    wo_d = nc.dram_tensor("wo", [d, d], BF16, kind="ExternalInput").ap()
    outT_d = nc.dram_tensor("outT", [b_loc, d, n], F32, kind="ExternalOutput").ap()

    with tile.TileContext(nc) as tc, ExitStack() as ctx, \
            nc.allow_low_precision(reason="bf16 attention by design"):
        # ---- pools ----
        const = ctx.enter_context(tc.tile_pool(name="const", bufs=1))
        xt_pool = ctx.enter_context(tc.tile_pool(name="xt", bufs=2))
        keep_pool = ctx.enter_context(tc.tile_pool(name="keep", bufs=2))
        qt_pool = ctx.enter_context(tc.tile_pool(name="qt", bufs=qtkt_bufs))
        kt_pool = ctx.enter_context(tc.tile_pool(name="kt", bufs=qtkt_bufs))
        vones_pool = ctx.enter_context(tc.tile_pool(name="vones", bufs=2))
        u_pool = ctx.enter_context(tc.tile_pool(name="u", bufs=u_bufs))
        uraw_pool = ctx.enter_context(tc.tile_pool(name="uraw", bufs=uraw_bufs))
        heads_pool = ctx.enter_context(tc.tile_pool(name="heads", bufs=2 * n_dc))
        outsb_pool = ctx.enter_context(tc.tile_pool(name="outsb", bufs=3))
        r_pool = ctx.enter_context(tc.tile_pool(name="r", bufs=3))
        rbc_pool = ctx.enter_context(tc.tile_pool(name="rbc", bufs=2))
        hnat_pool = ctx.enter_context(tc.tile_pool(name="hnat", bufs=2 * n_g))

        ps_s = ctx.enter_context(tc.tile_pool(name="ps_s", bufs=ps_s_bufs, space="PSUM"))
        ps_px = ctx.enter_context(tc.tile_pool(name="ps_px", bufs=ps_px_bufs, space="PSUM"))
        ps_av = ctx.enter_context(tc.tile_pool(name="ps_av", bufs=ps_av_bufs, space="PSUM"))

        # ---- constants: weights ----
        wq_sb = const.tile([P, n_dc, d], BF16, tag="wq")
        wk_sb = const.tile([P, n_dc, d], BF16, tag="wk")
        wv_sb = const.tile([P, n_dc, d], BF16, tag="wv")
        wo_sb = const.tile([P, n_dc, d], BF16, tag="wo")
        def load_w(w_sb, w_d, eng=None):
            (eng or nc.sync).dma_start(
                w_sb[:], w_d.rearrange("(kc p) e -> p kc e", p=P))

        # ---- per-batch state (emission-time bookkeeping) ----
        xt_t, keep_t, vones_t, qt_t, kt_t, heads_t, u_t = {}, {}, {}, {}, {}, {}, {}

        def load_xt(b, split=False):
            xt = xt_pool.tile([P, n_dc, n], BF16)
            src = qT_d[b].rearrange("(kc p) n -> p kc n", p=P)
            if split:
                hc = n_dc // 2
                nc.sync.dma_start(xt[:, 0:hc, :], src[:, 0:hc, :])
                xt_t[b] = (xt, src)
                return
            nc.sync.dma_start(xt[:], src)
            xt_t[b] = xt

        def load_xt_rest(b):
            xt, src = xt_t[b]
            hc = n_dc // 2
            nc.sync.dma_start(xt[:, hc:n_dc, :], src[:, hc:n_dc, :])
            xt_t[b] = xt

        def load_keep(b, half=None):
            if half in (None, 0):
                keep = keep_pool.tile([P, n_g, n], BF16, name="maskt")
                keep_t[b] = keep
            keep = keep_t[b]
            hg = n_g // 2
            src = mask_d[b].rearrange("(g p) n -> p g n", p=P)
            if half is None:
                nc.sync.dma_start(keep[:], src)
            elif half == 0:
                nc.sync.dma_start(keep[:, 0:hg, :], src[:, 0:hg, :])
            else:
                nc.sync.dma_start(keep[:, hg:n_g, :], src[:, hg:n_g, :])

        def alloc_batch(b):
            vones = vones_pool.tile([P, n_g, H * (KD + 1)], BF16)
            vh = vones[:].rearrange("p g (h x) -> p g h x", x=KD + 1)
            nc.gpsimd.memset(vh[:, :, :, KD:KD + 1], 1.0)
            vones_t[b] = vh
            heads_t[b] = [heads_pool.tile([P, n], BF16, tag="heads", name="heads")
                          for _ in range(n_dc)]
            qt_t[b] = {}
            kt_t[b] = {}

        def projqk_unit(b, kind, p, qh):
            w_sb = wq_sb if kind == 'q' else wk_sb
            dst_t, pool = (qt_t, qt_pool) if kind == 'q' else (kt_t, kt_pool)
            ps = ps_px.tile([P, 512], F32, tag="px")
            for kc in range(n_dc):
                nc.tensor.matmul(
                    ps[:], w_sb[:, kc, p * P:(p + 1) * P],
                    xt_t[b][:, kc, qh * 512:(qh + 1) * 512],
                    start=(kc == 0), stop=(kc == n_dc - 1))
            if p not in dst_t[b]:
                dst_t[b][p] = pool.tile([P, n], BF16, name=kind + "t")
            nc.vector.tensor_copy(dst_t[b][p][:, qh * 512:(qh + 1) * 512], ps[:])

        def projv_unit(b, g):
            ps = ps_px.tile([P, 512], F32, tag="px")
            for kc in range(n_dc):
                nc.tensor.matmul(
                    ps[:], xt_t[b][:, kc, g * P:(g + 1) * P], wv_sb[:, kc, :],
                    start=(kc == 0), stop=(kc == n_dc - 1))
            nc.vector.tensor_copy(
                vones_t[b][:, g, :, 0:KD],
                ps[:].rearrange("p (h x) -> p h x", x=KD))

        def out_unit(b, eb, qh, act_copy=False):
            ps = ps_px.tile([P, 512], F32, tag="px")
            for kc in range(n_dc):
                nc.tensor.matmul(
                    ps[:], wo_sb[:, kc, eb * P:(eb + 1) * P],
                    heads_t[b][kc][:, qh * 512:(qh + 1) * 512],
                    start=(kc == 0), stop=(kc == n_dc - 1))
            osb = outsb_pool.tile([P, 512], F32, tag="osb")
            # tail out-units copy on the (post-exp idle) ACT engine
            (nc.scalar.copy if act_copy else nc.vector.tensor_copy)(osb[:], ps[:])
            nc.sync.dma_start(
                outT_d[b, eb * P:(eb + 1) * P, qh * 512:(qh + 1) * 512], osb[:])

        _mask_ct = [0]

        def s_unit(b, p, g, hh):
            ps = ps_s.tile([P, n], F32, tag="s")
            rows = slice(hh * KD, (hh + 1) * KD)
            for qh in range(n_qh):
                qs = slice(qh * 512, (qh + 1) * 512)
                nc.tensor.matmul(
                    ps[:, qs], kt_t[b][p][rows, g * P:(g + 1) * P],
                    qt_t[b][p][rows, qs], start=True, stop=True,
                    tile_position=(hh * KD, 0))
            uraw = uraw_pool.tile([P, n], BF16, tag="uraw")
            nc.scalar.activation(uraw[:], ps[:], EXP)
            u = u_pool.tile([P, n], BF16, tag="u")
            _mask_ct[0] += 1
            eng = (nc.gpsimd if pool_mask_mod and _mask_ct[0] % pool_mask_mod == 0
                   else nc.vector)
            eng.tensor_mul(u[:], uraw[:], keep_t[b][:, g, :])
            u_t[(b, p, hh, g)] = u

        # ---- natural-layout AV: av_nat[q=128, v+1] per (head, q-chunk) ----
        hnat_t = {}

        def av_unit(b, p, hh, qc):
            h = 2 * p + hh
            av = ps_av.tile([P, KD + 1], F32, tag="avn", name="avn")
            for g in range(n_g):
                nc.tensor.matmul(
                    av[:], u_t[(b, p, hh, g)][:, qc * P:(qc + 1) * P],
                    vones_t[b][:, g, h, :],
                    start=(g == 0), stop=(g == n_g - 1))
            rinv = r_pool.tile([P, 1], F32, tag="rinv", name="rinv")
            nc.vector.reciprocal(rinv[:], av[:, KD:KD + 1])
            if (b, qc) not in hnat_t:
                hnat_t[(b, qc)] = hnat_pool.tile([P, d], BF16, name="hnat")
            nc.vector.tensor_scalar_mul(
                hnat_t[(b, qc)][:, h * KD:(h + 1) * KD], av[:, 0:KD], rinv[:])

        def tr_unit(b, p, qc):
            # last-pair transposes stay on SP: the ACT queue's end-of-program
            # drain would otherwise wait on ACT-issued DMA completions
            last = (b == b_loc - 1 and p == n_pairs - 1)
            eng = nc.scalar if (qc % 2 and not last) else nc.sync
            eng.dma_start(
                heads_t[b][p][:, qc * P:(qc + 1) * P],
                hnat_t[(b, qc)][:, p * P:(p + 1) * P], transpose=True)

        av_ps = {}

        def av_half(b, p, hh, qh, half):
            """AV accumulation split in two: half 0 covers g 0-3 (emittable
            once those u tiles exist), half 1 finishes + normalizes."""
            h = 2 * p + hh
            hv0 = h * KD
            if half == 0:
                av_ps[(b, p, hh, qh)] = ps_av.tile([KD + 1, 512], F32,
                                                   tag="av", name="av")
            av = av_ps[(b, p, hh, qh)]
            gs = range(n_g // 2) if half == 0 else range(n_g // 2, n_g)
            for g in gs:
                nc.tensor.matmul(
                    av[:], vones_t[b][:, g, h, :],
                    u_t[(b, p, hh, g)][:, qh * 512:(qh + 1) * 512],
                    start=(g == 0), stop=(g == n_g - 1))
            if half == 0:
                return
            r = r_pool.tile([1, 512], F32, tag="r", name="r")
            nc.vector.reciprocal(r[:], av[KD:KD + 1, :])
            rbc = rbc_pool.tile([KD, 512], F32, tag="rbcsb", name="rbcsb")
            nc.gpsimd.partition_broadcast(rbc[:], r[:])
            ht = heads_t[b][hv0 // P]
            nc.vector.tensor_mul(
                ht[hv0 % P:hv0 % P + KD, qh * 512:(qh + 1) * 512],
                av[0:KD, :], rbc[:])

        def attn(b, work, extra, late=()):
            """Score/exp/mask stream for batch b; `work` (AV of the previous
            pair/batch) and `extra` (projection/out closures) are popped at a
            bounded rate per score chunk.  `late` units are appended to the
            work queue one chunk per pair (after pair 0), keeping them
            ordered behind the previous batch's AV work."""
            late = list(late)
            n_late = max(1, (len(late) + n_pairs - 2) // max(1, n_pairs - 1)) \
                if late else 0
            for p in range(n_pairs):
                if p > 0:
                    work += late[:n_late]
                    del late[:n_late]
                for g in range(n_g):
                    s_unit(b, p, g, 0)
                    s_unit(b, p, g, 1)
                    if not nav and g == n_g // 2:
                        # first-half AV for this pair's hh tiles now exists
                        for qh2 in range(n_qh):
                            work.append(lambda pp=p, qh=qh2:
                                        av_half(b, pp, 0, qh, 0))
                    boost = 2 if (p == 0 and not extra) else 0
                    for _ in range(work_per_g + boost):
                        if work:
                            work.pop(0)()
                    for _ in range(extra_per_g):
                        if extra:
                            extra.pop(0)()
                        elif work:
                            work.pop(0)()
                while work:
                    work.pop(0)()
                if p == n_pairs - 1:
                    # next attn window (or the tail) needs every queued unit
                    # emitted before it starts popping this batch's AV work
                    while extra:
                        extra.pop(0)()
                if nav:
                    work += [(lambda pp=p, hh=hh2, qc=qc2, tr=tr:
                              (tr_unit(b, pp, qc) if tr
                               else av_unit(b, pp, hh, qc)))
                             for qc2 in range(n_g)
                             for hh2, tr in ((0, 0), (1, 0), (0, 1))]
                else:
                    work += [(lambda pp=p, hh=hh2, qh=qh2, hf=hf:
                              av_half(b, pp, hh, qh, hf))
                             for qh2 in range(n_qh)
                             for hh2, hf in ((0, 1), (1, 0), (1, 1))]
            return work

        loop_ctx = tc.For_i(0, hw_loop, 1) if hw_loop else contextlib.nullcontext()
        with loop_ctx:
          for _rep in range(max(1, repeat)):
            load_xt(0)
            if _rep == 0:
                load_w(wq_sb, wq_d)
                load_w(wk_sb, wk_d)
                load_w(wv_sb, wv_d)
            load_keep(0, half=0)
            if _rep == 0:
                load_w(wo_sb, wo_d)
            load_keep(0, half=1)
            load_xt(1)
            load_keep(1)
            if _rep == 0:
                # p-state warm-up: dummy matmuls while the first DMAs land
                # ramp the PE clock (0.65 -> 2.4 GHz after 3us busy)
                wu = outsb_pool.tile([P, 512], BF16, tag="warm", name="warm")
                nc.gpsimd.memset(wu[:], 0.0)
                ps_w = ps_s.tile([P, n], F32, tag="s", name="warmps")
                for i in range(14):
                    nc.tensor.matmul(ps_w[:, 0:512], wu[:, 0:P], wu[:],
                                     start=True, stop=True)
            alloc_batch(0)
            # prologue: pair-0 Q/K projections as two fat units on the ps_s
            # pool (one copy each) so the first S fires as early as possible
            for w_sb, dst_t, pool, kind in ((wq_sb, qt_t, qt_pool, "qt"),
                                            (wk_sb, kt_t, kt_pool, "kt")):
                ps = ps_s.tile([P, n], F32, tag="s", name="projps")
                for kc in range(n_dc):
                    for qh in range(n_qh):
                        nc.tensor.matmul(
                            ps[:, qh * 512:(qh + 1) * 512],
                            w_sb[:, kc, 0:P],
                            xt_t[0][:, kc, qh * 512:(qh + 1) * 512],
                            start=(kc == 0), stop=(kc == n_dc - 1))
                dst_t[0][0] = pool.tile([P, n], BF16, name=kind)
                nc.vector.tensor_copy(dst_t[0][0][:], ps[:])
            # attention b0: V + remaining b0 projections, then all of b1's
            extra = [lambda g=g: projv_unit(0, g) for g in range(n_g)]
            for p in range(1, n_pairs):
                for kind in ('q', 'k'):
                    for qh in range(n_qh):
                        extra.append(lambda b=0, k=kind, pp=p, qh=qh:
                                     projqk_unit(b, k, pp, qh))
            extra.append(lambda: alloc_batch(1))
            for p in range(n_pairs):
                for kind in ('q', 'k'):
                    for qh in range(n_qh):
                        extra.append(lambda b=1, k=kind, pp=p, qh=qh:
                                     projqk_unit(b, k, pp, qh))
            for g in range(n_g):
                extra.append(lambda b=1, g=g: projv_unit(b, g))
            work = attn(0, [], extra)
            # attention b1: out-projection of b0 under b1's exp stream.
            # Out units are `late` work: appended to the ordered work queue
            # (one chunk per pair, after pair 0) BEHIND b0's final AV +
            # transpose units — they read b0's heads tiles, which those units
            # produce, and a reader emitted before its writer would invert
            # the tracked dependency.
            late = [lambda eb=eb, qh=qh: out_unit(0, eb, qh)
                    for qh in range(n_qh) for eb in range(n_dc)]
            work = attn(1, work, [], late=late)
            # epilogue: drain b1-pair3 AV qh-half by qh-half, interleaving the
            # qh0 out-projection with the qh1 AV/transpose drain so the last
            # transposes complete under the out units' PE work
            half = 3 * (n_g // 2)
            for w in work[:half]:
                w()
            rest = work[half:]
            step = max(1, len(rest) // n_dc)
            for eb in range(n_dc):
                out_unit(1, eb, 0, act_copy=True)
                for w in rest[eb * step:(eb + 1) * step]:
                    w()
            for w in rest[n_dc * step:]:
                w()
            ps_f = ps_px.tile([P, 512], F32, tag="px", name="fillps")
            for i in range(4):  # hold the PE p-state through the DMA waits
                nc.tensor.matmul(ps_f[:], wu[:, 0:P], wu[:],
                                 start=True, stop=True)
            for eb in range(n_dc):
                out_unit(1, eb, 1, act_copy=True)

    nc.compile()
    return nc


def prep_inputs(q, mask, W_query, W_key, W_val, W_out):
    """Host-side prep: transpose/cast to the kernel's bf16 layouts and build
    the per-core input maps."""
    scale = np.float32(1.0 / np.sqrt(KD))
    qT = np.ascontiguousarray(
        q.transpose(0, 2, 1)).astype(ml_dtypes.bfloat16)
    maskT = np.ascontiguousarray(
        (~mask).transpose(0, 2, 1)).astype(ml_dtypes.bfloat16)
    wq = np.ascontiguousarray(
        (W_query * scale).transpose(1, 0, 2).reshape(D, H * KD)).astype(
            ml_dtypes.bfloat16)
    wk = np.ascontiguousarray(
        W_key.transpose(1, 0, 2).reshape(D, H * KD)).astype(ml_dtypes.bfloat16)
    wv = np.ascontiguousarray(
        W_val.transpose(1, 0, 2).reshape(D, H * KD)).astype(ml_dtypes.bfloat16)
    wo = np.ascontiguousarray(
        W_out.reshape(H * KD, D)).astype(ml_dtypes.bfloat16)
    return [
        {
            "qT": qT[c * B_LOC:(c + 1) * B_LOC],
            "maskT": maskT[c * B_LOC:(c + 1) * B_LOC],
            "wq": wq, "wk": wk, "wv": wv, "wo": wo,
        }
        for c in range(NCORES)
    ]


def _get_nc(key=(B_LOC, N)):
    if key not in _NC_CACHE:
        _NC_CACHE[key] = build_attention_nc(*key)
    return _NC_CACHE[key]


def kernel(q, mask, W_query, W_key, W_val, W_out):
    from concourse.bass_utils import run_bass_kernel_spmd

    in_maps = prep_inputs(q, mask, W_query, W_key, W_val, W_out)
    nc = _get_nc()
    last_exc = None
    for attempt in range(3):
        try:
            res = run_bass_kernel_spmd(nc, in_maps, core_ids=list(range(NCORES)))
            break
        except Exception as e:  # transient NRT device wedge -> retry
            last_exc = e
            import time as _time
            _time.sleep(5 * (attempt + 1))
    else:
        raise last_exc
    outT = np.concatenate([r["outT"] for r in res.results], axis=0)  # (16, 512, 1024)
    return np.ascontiguousarray(outT.transpose(0, 2, 1), dtype=np.float32)
